# revision 38
# baseline (speedup 1.0000x reference)
"""Trainium2 Bass kernel for a pre-LN transformer encoder layer.

Model: D_MODEL=1024, N_HEADS=16, D_K=64, D_FF=4096, B=2, S=2048, fp32 I/O.

Sharding: fully data-parallel over 8 cores = (batch b, query-block j) with
512 query tokens per core.  Each core recomputes LN1/K/V for its full batch
element (no collectives), computes attention + FFN for its own 512 tokens,
and writes its [512, 1024] slice of the output.  Per-core inputs are rotated
so the core's own query block is always tokens [0:512).

v2 (pipelined): emission is interleaved so the ACT exp stream (the hard
floor: 16.8M exps = ~110us) starts as soon as Q + the first key-group's K
are projected; the remaining LN1/K/V work is emitted in "growth" stages
under head-pair 0's score loop.  The two heads of a pair share one
[128,1024] PSUM score tile so a single Exp instruction covers both (the
mask bias is per-partition = per-key, identical for both heads).  E is
quartered so next-pair exps overlap current-pair AV accumulation.  The
softmax reciprocal is broadcast with a tiny selector matmul instead of a
DRAM round-trip.  FFN1 runs in fp8 DoubleRow (w1 fp8, ~1.2e-2 rel err);
FFN2 stays bf16 and streams w2 in [128,512] chunks interleaved with FFN1.

Host-side preprocessing (exact linear-algebra folds):
  - LN1 affine folded into wq/wk/wv;  1/sqrt(d_k) folded into wq
  - V bias folded into the O-projection bias (softmax rows sum to 1)
  - LN2 affine folded into w1
"""

import os
import sys

sys.path.insert(0, "/opt/trn_rl_repo")

import numpy as np
import ml_dtypes

import concourse.bass as bass
import concourse.tile as tile
from concourse import bacc, mybir
from concourse.bass_utils import run_bass_kernel_spmd
from concourse.masks import make_identity

F32 = mybir.dt.float32
BF16 = mybir.dt.bfloat16
FP8 = mybir.dt.float8e4
DR = mybir.MatmulPerfMode.DoubleRow
AF = mybir.ActivationFunctionType
ALU = mybir.AluOpType

# host-side fp8 weight scales (avoid e4m3 subnormals); undone on device
SQ = 256.0        # wq (has 1/sqrt(dk) folded, sigma ~0.0025)
SK = 64.0         # wk
SV = 64.0         # wv
SO = 64.0         # wo
SW1 = 128.0       # w1 (ln2_w folded, sigma ~0.02)
IS = 1.0 / 16.0   # Ou pre-scale: keeps |ops| (sigma~34, tails >240) in fp8 range
RB = 1024.0       # recip scale; IS*RB = 64 = effective attention-out scale
OS = IS * RB

D = 1024          # d_model
H = 16            # heads
DK = 64           # head dim
DFF = 4096        # ffn hidden
S = 2048          # keys per batch element (per core)
Q = 512           # query tokens per core
EPS = 1e-6
NCHIP = 8
VS = 68           # V slot stride per head (64 data + ones col + 3 pad, 4B aligned)

NT = S // 128     # 16 token tiles of the full batch element
NQ = Q // 128     # 4 token tiles of own block
NR = D // 128     # 8 feature chunks of d_model
NF = DFF // 128   # 32 feature chunks of d_ff

GP = os.environ.get("BASS_GPSIMD", "0") == "1"   # gpsimd offload of LN normalize
# FFN1 in fp8 DoubleRow: w1 is sent two-level (hi + residual, both fp8) so its
# quantization error vanishes; the fp8 LN2 output g8 dominates (~1.2e-2 rel).
FFN1_FP8 = os.environ.get("BASS_FFN1_FP8", "0") == "1"
# FFN2 in fp8 DoubleRow: gelu writes H1T fp8 directly, w2 sent fp8(x128).
# Adds ~1.1e-2 (h1) (+) ~1.2e-2 (w2) in quadrature => ~1.67e-2 total rel err.
FFN2_FP8 = os.environ.get("BASS_FFN2_FP8", "1") == "1"
SW2 = 128.0


def _build():
    nc = bacc.Bacc("TRN2", target_bir_lowering=False, debug=False)

    x_all = nc.dram_tensor("x_all", [S, D], F32, kind="ExternalInput").ap()
    wq_d = nc.dram_tensor("wq_b", [D, D], FP8, kind="ExternalInput").ap()
    wk_d = nc.dram_tensor("wk_b", [D, D], FP8, kind="ExternalInput").ap()
    wv_d = nc.dram_tensor("wv_b", [D, D], FP8, kind="ExternalInput").ap()
    wo_d = nc.dram_tensor("wo_b", [D, D], FP8, kind="ExternalInput").ap()
    w1_d = nc.dram_tensor("w1_b", [D, DFF], FP8 if FFN1_FP8 else BF16,
                          kind="ExternalInput").ap()
    w1r_d = (nc.dram_tensor("w1r_b", [D, DFF], FP8, kind="ExternalInput").ap()
             if FFN1_FP8 else None)
    w2_d = nc.dram_tensor("w2_b", [DFF, D], FP8 if FFN2_FP8 else BF16,
                          kind="ExternalInput").ap()
    bq_d = nc.dram_tensor("bq_v", [D], F32, kind="ExternalInput").ap()
    bk_d = nc.dram_tensor("bk_v", [D], F32, kind="ExternalInput").ap()
    bo_d = nc.dram_tensor("bo_v", [D], F32, kind="ExternalInput").ap()
    b1_d = nc.dram_tensor("b1_v", [DFF], F32, kind="ExternalInput").ap()
    b2_d = nc.dram_tensor("b2_v", [D], F32, kind="ExternalInput").ap()
    mb_d = nc.dram_tensor("mb_v", [S], F32, kind="ExternalInput").ap()
    sel_d = nc.dram_tensor("sel_v", [16, NR * 128], FP8, kind="ExternalInput").ap()
    out_d = nc.dram_tensor("out", [Q, D], F32, kind="ExternalOutput").ap()

    with tile.TileContext(nc) as tc:
        _emit(nc, tc, x_all, wq_d, wk_d, wv_d, wo_d, w1_d, w1r_d, w2_d,
              bq_d, bk_d, bo_d, b1_d, b2_d, mb_d, sel_d, out_d)
    nc.compile()
    return nc


def _emit(nc, tc, *args):
    R = int(os.environ.get("BASS_REPEAT", "1"))
    if R > 1:
        with tc.For_i(0, R, 1):
            _emit_body(nc, tc, *args)
    else:
        _emit_body(nc, tc, *args)


def _emit_body(nc, tc, x_all, wq_d, wk_d, wv_d, wo_d, w1_d, w1r_d, w2_d,
               bq_d, bk_d, bo_d, b1_d, b2_d, mb_d, sel_d, out_d):
    PHASES = os.environ.get("BASS_PHASES", "ABCDE")

    # ---- constant / bias staging ---------------------------------------
    consts = tc.alloc_tile_pool(name="consts", bufs=1)
    ident_bf = consts.tile([128, 128], BF16)
    make_identity(nc, ident_bf)
    bq_sb = consts.tile([128, NR], F32)
    nc.sync.dma_start(bq_sb[:], bq_d.rearrange("(m p) -> p m", p=128))
    # (bk is dropped entirely: K-bias only adds a per-query constant to the
    # scores, and softmax is invariant to it — exact.)
    mb_sb = consts.tile([128, NT], F32)
    nc.sync.dma_start(mb_sb[:], mb_d.rearrange("(i p) -> p i", p=128))
    # late-needed consts: tiles allocated now, DMAs emitted later so the
    # startup queue is x tiles + wq/wk only
    bo_sb = consts.tile([128, NR], F32)
    b1_sb = consts.tile([128, NF], F32)
    sel_sb = consts.tile([16, NR * 128], FP8)
    b2b = consts.tile([128, D], F32)

    psum = tc.alloc_tile_pool(name="psum", bufs=1, space="PSUM")

    # ---- long-lived SBUF tensors ---------------------------------------
    poolF = tc.alloc_tile_pool(name="poolF", bufs=1)
    if FFN1_FP8:
        w1_sb = poolF.tile([128, NR * DFF], FP8)      # din chunk r at [r*DFF, +DFF)
        w1r_sb = poolF.tile([128, NR * DFF], FP8)     # fp8 residual of w1
    else:
        w1_sb = poolF.tile([128, NR * DFF], BF16)
        w1r_sb = None

    poolQKV = tc.alloc_tile_pool(name="poolQKV", bufs=1)
    QT = poolQKV.tile([128, NR * Q], FP8)             # chunk m at [m*Q, +Q)
    KT = poolQKV.tile([128, NR * S], FP8)             # chunk m at [m*S, +S)
    V_sb = poolQKV.tile([128, NT * 16 * VS], FP8)     # tok tile t at t*16*VS, head h at +h*VS
    Ou = poolQKV.tile([128, NR * Q], FP8)             # attn out * IS (later *recip*RB)
    sums_st = poolQKV.tile([128, 4 * Q], F32)         # head h at (32*(h//4), (h%4)*Q)
    sums16 = poolQKV.tile([16, Q], F32)
    recipb = poolQKV.tile([16, Q], FP8)
    mvv = poolQKV.tile([128, NT, 2], F32)             # LN1 (mean, var) per tile
    rstd_all = poolQKV.tile([128, NT], F32)           # LN1 1/(std+eps) per tile

    poolE = tc.alloc_tile_pool(name="poolE", bufs=1)
    # quarter qt holds key tiles i=4qt..4qt+3; per i: head-A cols then head-B cols
    Eq = [poolE.tile([128, 4 * 2 * Q], FP8, name=f"Eq{k}") for k in range(4)]

    poolA = tc.alloc_tile_pool(name="poolA", bufs=1, side="right")
    ln1T = poolA.tile([128, NR * S], FP8)             # chunk r at [r*S, +S)
    poolWqkv = tc.alloc_tile_pool(name="poolWqkv", bufs=1, side="right")
    wq_sb = poolWqkv.tile([128, NR * D], FP8)
    wk_sb = poolWqkv.tile([128, NR * D], FP8)
    wv_sb = poolWqkv.tile([128, NR * D], FP8)
    poolWo = tc.alloc_tile_pool(name="poolWo", bufs=1, side="right")
    wo_sb = poolWo.tile([128, NR * D], FP8)
    streamA = tc.alloc_tile_pool(name="streamA", bufs=3, side="right")

    Vv = V_sb.rearrange("p (t h s) -> p t h s", t=NT, s=VS)
    V3 = V_sb.rearrange("p (t x) -> p t x", t=NT)
    nc.vector.memset(Vv[:, :, :, 64:65], 1.0)        # ones column for row-sums

    wq3 = wq_sb.rearrange("p (r d) -> p r d", r=NR)
    wk3 = wk_sb.rearrange("p (r d) -> p r d", r=NR)
    wv3 = wv_sb.rearrange("p (r d) -> p r d", r=NR)
    wo3 = wo_sb.rearrange("p (r d) -> p r d", r=NR)
    w13 = w1_sb.rearrange("p (r f) -> p r f", r=NR)
    w1r3 = w1r_sb.rearrange("p (r f) -> p r f", r=NR) if FFN1_FP8 else None
    ln3 = ln1T.rearrange("p (r s) -> p r s", r=NR)
    Ou3 = Ou.rearrange("p (r q) -> p r q", r=NR)

    def _dummy_out():
        nc.sync.dma_start(out_d[:, :], x_all[0:Q, :])

    # ---- pipeline building blocks --------------------------------------
    def ln_stats(t, keep=False):
        """bn_stats/aggr of token tile t into mvv; returns xt if keep."""
        xt = streamA.tile([128, D], F32, bufs=4, name="xt")
        nc.sync.dma_start(xt[:], x_all[t * 128:(t + 1) * 128, :])
        stats = streamA.tile([128, 2, 6], F32, bufs=4, name="stats")
        xg = xt.rearrange("p (g d) -> p g d", g=2)
        nc.vector.bn_stats(stats[:, 0, :], xg[:, 0, :])
        nc.vector.bn_stats(stats[:, 1, :], xg[:, 1, :])
        nc.vector.bn_aggr(mvv[:, t, :], stats[:])
        return xt if keep else None

    def ln_rstd(t0, t1):
        """Batched 1/(std+eps) for tiles [t0, t1) — one ACT Sqrt."""
        # std with Bessel correction (ddof=1), then 1/(std+eps)
        nc.scalar.activation(rstd_all[:, t0:t1], mvv[:, t0:t1, 1],
                             AF.Sqrt, scale=float(D) / (D - 1))
        nc.vector.tensor_scalar_add(rstd_all[:, t0:t1], rstd_all[:, t0:t1], EPS)
        nc.vector.reciprocal(rstd_all[:, t0:t1], rstd_all[:, t0:t1])

    def ln_tile(t, xt=None):
        """Normalize tile t (stats precomputed) -> transpose into ln1T."""
        if xt is None:
            xt = streamA.tile([128, D], F32, bufs=4, name="xt")
            nc.sync.dma_start(xt[:], x_all[t * 128:(t + 1) * 128, :])
        lt = streamA.tile([128, D], BF16, bufs=3, name="lt")
        eng = nc.gpsimd if GP else nc.vector
        eng.tensor_scalar(
            out=lt[:], in0=xt[:], scalar1=mvv[:, t, 0:1], scalar2=rstd_all[:, t:t + 1],
            op0=ALU.subtract, op1=ALU.mult)
        for half in range(2):
            tp = psum.tile([128, 512], BF16, tag="mm", bufs=2, name="trp")
            for j in range(4):
                r = 4 * half + j
                nc.tensor.transpose(
                    tp[:, j * 128:(j + 1) * 128],
                    lt[:, r * 128:(r + 1) * 128], ident_bf[:])
            dst = ln3[:, 4 * half:4 * half + 4, t * 128:(t + 1) * 128]
            src = tp.rearrange("p (r c) -> p r c", r=4)
            nc.vector.tensor_copy(dst, src)

    def _apply(idx, out, in0, sc, bias_ap):
        """PSUM->SBUF scale+bias; mostly DVE (ACT has ~600ns/inst overhead)."""
        if idx % 3 != 2:
            nc.vector.tensor_scalar(
                out=out, in0=in0, scalar1=sc, scalar2=bias_ap,
                op0=ALU.mult, op1=ALU.add)
        else:
            nc.scalar.activation(out, in0, AF.Identity, bias=bias_ap, scale=sc)

    def q_chunk(m):
        qps = psum.tile([128, Q], F32, tag="mm", bufs=2, name="qps")
        for rq in range(NR // 2):
            nc.tensor.matmul(
                qps[:], wq3[:, 2 * rq:2 * rq + 2, m * 128:(m + 1) * 128],
                ln3[:, 2 * rq:2 * rq + 2, 0:Q], perf_mode=DR,
                start=(rq == 0), stop=(rq == NR // 2 - 1))
        _apply(m, QT[:, m * Q:(m + 1) * Q], qps[:], 1.0 / SQ, bq_sb[:, m:m + 1])

    def k_group(g, ms=tuple(range(NR))):
        for m in ms:
            kps = psum.tile([128, Q], F32, tag="mm", bufs=2, name="kps")
            for rq in range(NR // 2):
                nc.tensor.matmul(
                    kps[:], wk3[:, 2 * rq:2 * rq + 2, m * 128:(m + 1) * 128],
                    ln3[:, 2 * rq:2 * rq + 2, g * Q:(g + 1) * Q], perf_mode=DR,
                    start=(rq == 0), stop=(rq == NR // 2 - 1))
            _apply(m, KT[:, m * S + g * Q: m * S + (g + 1) * Q], kps[:],
                   1.0 / SK, 0.0)

    def v_tile(t):
        for s2 in range(2):
            vps = psum.tile([128, Q], F32, tag="mm", bufs=2, name="vps")
            for rq in range(NR // 2):
                nc.tensor.matmul(
                    vps[:], ln3[:, 2 * rq:2 * rq + 2, t * 128:(t + 1) * 128],
                    wv3[:, 2 * rq:2 * rq + 2, s2 * Q:(s2 + 1) * Q], perf_mode=DR,
                    start=(rq == 0), stop=(rq == NR // 2 - 1))
            _apply(t + s2, Vv[:, t, s2 * 8:(s2 + 1) * 8, 0:64],
                   vps.rearrange("p (h d) -> p h d", d=64), 1.0 / SV, 0.0)

    def scores_exp(m, i):
        sps = psum.tile([128, 2 * Q], F32, tag="sc", bufs=2, name="sps")
        ks = m * S + i * 128
        nc.tensor.matmul(
            sps[:, 0:Q], KT[0:64, ks:ks + 128],
            QT[0:64, m * Q:(m + 1) * Q], start=True, stop=True,
            skip_group_check=True)
        nc.tensor.matmul(
            sps[:, Q:2 * Q], KT[64:128, ks:ks + 128],
            QT[64:128, m * Q:(m + 1) * Q], start=True, stop=True,
            skip_group_check=True)
        nc.scalar.activation(
            Eq[i // 4][:, (i % 4) * 2 * Q:(i % 4 + 1) * 2 * Q], sps[:],
            AF.Exp, bias=mb_sb[:, i:i + 1], scale=1.0)

    def av_pass(m, k, opsA, opsB):
        Eqt = Eq[k // 2].rearrange("p (i c) -> p i c", c=2 * Q)
        ii = (2 * k) % 4
        nc.tensor.matmul(
            opsA[0:65, :], V3[:, 2 * k:2 * k + 2, (2 * m) * VS:(2 * m) * VS + 65],
            Eqt[:, ii:ii + 2, 0:Q], perf_mode=DR,
            start=(k == 0), stop=(k == 7), skip_group_check=True)
        nc.tensor.matmul(
            opsB[0:65, :], V3[:, 2 * k:2 * k + 2, (2 * m + 1) * VS:(2 * m + 1) * VS + 65],
            Eqt[:, ii:ii + 2, Q:2 * Q], perf_mode=DR,
            start=(k == 0), stop=(k == 7), skip_group_check=True)

    def stage_sums(m, opsA, opsB):
        for h, ops in ((2 * m, opsA), (2 * m + 1, opsB)):
            p4, c4 = 32 * (h // 4), (h % 4) * Q
            nc.vector.tensor_copy(sums_st[p4:p4 + 1, c4:c4 + Q], ops[64:65, :])
        nc.vector.tensor_scalar(
            out=Ou[0:64, m * Q:(m + 1) * Q], in0=opsA[0:64, :],
            scalar1=IS, scalar2=0.0, op0=ALU.mult, op1=ALU.add)
        nc.vector.tensor_scalar(
            out=Ou[64:128, m * Q:(m + 1) * Q], in0=opsB[0:64, :],
            scalar1=IS, scalar2=0.0, op0=ALU.mult, op1=ALU.add)

    # ---- prologue: LN1 stats, first key group, Q -----------------------
    xts = [ln_stats(t, keep=True) for t in range(4)]
    ln_rstd(0, 4)
    for r in range(NR):
        nc.sync.dma_start(wq_sb[:, r * D:(r + 1) * D], wq_d[r * 128:(r + 1) * 128, :])
    for r in range(NR):
        nc.sync.dma_start(wk_sb[:, r * D:(r + 1) * D], wk_d[r * 128:(r + 1) * 128, :])
    for t in range(4):
        ln_tile(t, xts[t])
    q_chunk(0)
    k_group(0, ms=(0,))
    for t in range(4, NT):
        ln_stats(t)
    for r in range(NR):
        nc.sync.dma_start(wv_sb[:, r * D:(r + 1) * D], wv_d[r * 128:(r + 1) * 128, :])
    ln_rstd(4, NT)

    # ---- attention: head pairs, growth front-loaded under pair 0 -------
    for m in range(NR):
        opsA = psum.tile([128, Q], F32, tag="av", bufs=2, name="avA")
        opsB = psum.tile([128, Q], F32, tag="av", bufs=2, name="avB")
        for i in range(NT):
            if m == 0:
                # growth: LN tiles, pair-0 K groups, and V spread through
                # pair 0's window; AV(0) is deferred past the V projections
                if i == 0:
                    for t in range(4, 8):
                        ln_tile(t)
                    k_group(1, ms=(0,))
                elif i == 2:
                    for t in range(8, 12):
                        ln_tile(t)
                elif i == 3:
                    k_group(2, ms=(0,))
                elif i == 5:
                    for t in range(12, 16):
                        ln_tile(t)
                elif i == 6:
                    k_group(3, ms=(0,))
                if i >= 2 and i < 10:
                    v_tile(2 * (i - 2))
                    v_tile(2 * (i - 2) + 1)
                if i == 13:
                    # late-needed weights; queued after all x/wqkv traffic
                    for r in range(NR):
                        nc.sync.dma_start(wo_sb[:, r * D:(r + 1) * D],
                                          wo_d[r * 128:(r + 1) * 128, :])
                    for r in range(NR):
                        nc.sync.dma_start(w1_sb[:, r * DFF:(r + 1) * DFF],
                                          w1_d[r * 128:(r + 1) * 128, :])
                    if FFN1_FP8:
                        for r in range(NR):
                            nc.sync.dma_start(w1r_sb[:, r * DFF:(r + 1) * DFF],
                                              w1r_d[r * 128:(r + 1) * 128, :])
            scores_exp(m, i)
            if m < NR - 1 and i % 4 == 1:
                # pair m+1's K chunks (one group per quarter) and Q chunk:
                # keeps the PE stream dense through phase C (HAM full clock)
                k_group(i // 4, ms=(m + 1,))
                if i == 1:
                    q_chunk(m + 1)
            if m > 0 and i % 2 == 1:
                av_pass(m, i // 2, opsA, opsB)
        if m == 0:
            for k in range(8):
                av_pass(m, k, opsA, opsB)
            streamA.release()
            # late consts ahead of their consumers (sel/bo for recip/O-proj,
            # b2b/b1 for D/FFN)
            nc.sync.dma_start(sel_sb[:], sel_d[:, :])
            nc.sync.dma_start(bo_sb[:], bo_d.rearrange("(m p) -> p m", p=128))
            nc.sync.dma_start(b1_sb[:], b1_d.rearrange("(f p) -> p f", p=128))
            b2_bc = bass.AP(tensor=b2_d.tensor, offset=b2_d.offset,
                            ap=[[0, 128]] + list(b2_d.ap))
            nc.sync.dma_start(b2b[:], b2_bc)
        stage_sums(m, opsA, opsB)

    # ---- softmax normalization (bulk recip + selector broadcast) -------
    sums_src = sums_st.rearrange("(a b) (h q) -> a b h q", b=32, h=4)[:, 0, :, :]
    nc.sync.dma_start(sums16[:, :], sums_src)
    nc.vector.reciprocal(sums16[:], sums16[:])
    nc.vector.tensor_scalar(
        out=recipb[:], in0=sums16[:], scalar1=RB, scalar2=0.0,
        op0=ALU.mult, op1=ALU.add)
    for m in range(NR):
        rbp = psum.tile([128, Q], F32, tag="mm", bufs=2, name="rbp")
        nc.tensor.matmul(rbp[:], sel_sb[:, m * 128:(m + 1) * 128], recipb[:],
                         start=True, stop=True)
        nc.vector.tensor_tensor(
            out=Ou[:, m * Q:(m + 1) * Q],
            in0=Ou[:, m * Q:(m + 1) * Q], in1=rbp[:], op=ALU.mult)

    poolE.release()
    if "D" not in PHASES:
        _dummy_out()
        poolWo.release(); poolWqkv.release(); poolA.release()
        poolQKV.release(); poolF.release(); consts.release(); psum.release()
        return

    # ---- O-projection + residual + LN2 ---------------------------------
    poolD = tc.alloc_tile_pool(name="poolD", bufs=1)
    mhaT = poolD.tile([128, NR * Q], BF16)
    h_sb = poolD.tile([128, NQ * D], F32)
    gT = poolD.tile([128, NR * Q], FP8 if FFN1_FP8 else BF16)
    gT3 = gT.rearrange("p (r q) -> p r q", r=NR)

    for mo in range(NR):
        mps = psum.tile([128, Q], F32, tag="mm", bufs=2, name="mps")
        for rq in range(NR // 2):
            nc.tensor.matmul(
                mps[:], wo3[:, 2 * rq:2 * rq + 2, mo * 128:(mo + 1) * 128],
                Ou3[:, 2 * rq:2 * rq + 2, :], perf_mode=DR,
                start=(rq == 0), stop=(rq == NR // 2 - 1))
        _apply(mo, mhaT[:, mo * Q:(mo + 1) * Q], mps[:],
               1.0 / (SO * OS), bo_sb[:, mo:mo + 1])
    poolWo.release()
    poolWqkv.release()
    poolA.release()

    streamD = tc.alloc_tile_pool(name="streamD", bufs=2, side="right")
    mv2v = streamD.tile([128, NQ, 2], F32, bufs=1, name="mv2v")
    rstd2 = streamD.tile([128, NQ], F32, bufs=1, name="rstd2")
    for t in range(NQ):
        xo = streamD.tile([128, D], F32, bufs=4, name="xo")
        nc.sync.dma_start(xo[:], x_all[t * 128:(t + 1) * 128, :])
        for half in range(2):
            tp = psum.tile([128, 512], BF16, tag="mm", bufs=2, name="trd")
            for j in range(4):
                r = 4 * half + j
                nc.tensor.transpose(
                    tp[:, j * 128:(j + 1) * 128],
                    mhaT[:, r * Q + t * 128: r * Q + (t + 1) * 128], ident_bf[:])
            nc.vector.tensor_tensor(
                out=h_sb[:, t * D + half * 512: t * D + (half + 1) * 512],
                in0=tp[:], in1=xo[:, half * 512:(half + 1) * 512], op=ALU.add)
        ht = h_sb[:, t * D:(t + 1) * D]
        stats2 = streamD.tile([128, 2, 6], F32, bufs=4, name="st2")
        hg = ht.rearrange("p (g d) -> p g d", g=2)
        nc.vector.bn_stats(stats2[:, 0, :], hg[:, 0, :])
        nc.vector.bn_stats(stats2[:, 1, :], hg[:, 1, :])
        nc.vector.bn_aggr(mv2v[:, t, :], stats2[:])
    nc.scalar.activation(rstd2[:], mv2v[:, :, 1], AF.Sqrt, scale=float(D) / (D - 1))
    nc.vector.tensor_scalar_add(rstd2[:], rstd2[:], EPS)
    nc.vector.reciprocal(rstd2[:], rstd2[:])
    for t in range(NQ):
        ht = h_sb[:, t * D:(t + 1) * D]
        gt_ = streamD.tile([128, D], BF16, bufs=2, name="gt")
        nc.vector.tensor_scalar(
            out=gt_[:], in0=ht, scalar1=mv2v[:, t, 0:1], scalar2=rstd2[:, t:t + 1],
            op0=ALU.subtract, op1=ALU.mult)
        for half in range(2):
            tg = psum.tile([128, 512], BF16, tag="mm", bufs=2, name="trg")
            for j in range(4):
                r = 4 * half + j
                nc.tensor.transpose(
                    tg[:, j * 128:(j + 1) * 128],
                    gt_[:, r * 128:(r + 1) * 128], ident_bf[:])
            dst = gT3[:, 4 * half:4 * half + 4, t * 128:(t + 1) * 128]
            src = tg.rearrange("p (r c) -> p r c", r=4)
            nc.vector.tensor_copy(dst, src)
        # h += b2 after g extracted (gpsimd: DVE-free)
        eng2 = nc.gpsimd if GP else nc.vector
        eng2.tensor_tensor(out=ht, in0=ht, in1=b2b[:], op=ALU.add)
    streamD.release()
    if "E" not in PHASES:
        _dummy_out()
        poolD.release()
        poolQKV.release(); poolF.release(); consts.release(); psum.release()
        return

    # ---- FFN: FFN1 fp8 DR interleaved with FFN2 wave A, then wave B ----
    poolFF = tc.alloc_tile_pool(name="poolFF", bufs=1)
    H1T = poolFF.tile([128, NF * Q], FP8 if FFN2_FP8 else BF16)
    H1T3 = H1T.rearrange("p (f q) -> p f q", f=NF)
    streamW2 = tc.alloc_tile_pool(name="streamW2", bufs=6)
    streamE = tc.alloc_tile_pool(name="streamE", bufs=2)

    W2DT = FP8 if FFN2_FP8 else BF16

    def ffn1_chunk(f):
        fps = psum.tile([128, Q], F32, tag="sc", bufs=2, name="fps")
        if FFN1_FP8:
            for rq in range(NR // 2):
                nc.tensor.matmul(
                    fps[:], w13[:, 2 * rq:2 * rq + 2, f * 128:(f + 1) * 128],
                    gT3[:, 2 * rq:2 * rq + 2, :], perf_mode=DR,
                    start=(rq == 0), stop=False)
            for rq in range(NR // 2):
                nc.tensor.matmul(
                    fps[:], w1r3[:, 2 * rq:2 * rq + 2, f * 128:(f + 1) * 128],
                    gT3[:, 2 * rq:2 * rq + 2, :], perf_mode=DR,
                    start=False, stop=(rq == NR // 2 - 1))
        else:
            for r in range(NR):
                nc.tensor.matmul(
                    fps[:], w13[:, r, f * 128:(f + 1) * 128],
                    gT3[:, r, :], start=(r == 0), stop=(r == NR - 1))
        nc.scalar.activation(
            H1T[:, f * Q:(f + 1) * Q], fps[:], AF.Gelu,
            bias=b1_sb[:, f:f + 1], scale=(1.0 / SW1) if FFN1_FP8 else 1.0)

    def ffn2_wave(s2, with_ffn1):
        ops2 = [psum.tile([128, Q], F32, tag=tg, bufs=2, name=f"ff{s2}{t}")
                for t, tg in ((0, "av"), (1, "av"), (2, "mm"), (3, "mm"))]
        if FFN2_FP8:
            if with_ffn1:
                ffn1_chunk(0)
                ffn1_chunk(1)
            for fp in range(NF // 2):
                if with_ffn1 and fp + 1 < NF // 2:
                    ffn1_chunk(2 * fp + 2)
                    ffn1_chunk(2 * fp + 3)
                w2p = streamW2.tile([128, 2 * Q], W2DT, bufs=4, name="w2s")
                nc.sync.dma_start(
                    w2p[:, 0:Q], w2_d[(2 * fp) * 128:(2 * fp + 1) * 128,
                                      s2 * Q:(s2 + 1) * Q])
                nc.sync.dma_start(
                    w2p[:, Q:2 * Q], w2_d[(2 * fp + 1) * 128:(2 * fp + 2) * 128,
                                          s2 * Q:(s2 + 1) * Q])
                w2p3 = w2p.rearrange("p (j q) -> p j q", j=2)
                for t in range(NQ):
                    nc.tensor.matmul(
                        ops2[t][:], H1T3[:, 2 * fp:2 * fp + 2, t * 128:(t + 1) * 128],
                        w2p3[:, :, :], perf_mode=DR,
                        start=(fp == 0), stop=(fp == NF // 2 - 1),
                        skip_group_check=True)
        else:
            for f in range(NF):
                if with_ffn1:
                    ffn1_chunk(f)
                w2s = streamW2.tile([128, Q], W2DT, bufs=6, name="w2s")
                nc.sync.dma_start(w2s[:], w2_d[f * 128:(f + 1) * 128,
                                               s2 * Q:(s2 + 1) * Q])
                for t in range(NQ):
                    nc.tensor.matmul(
                        ops2[t][:], H1T[:, f * Q + t * 128: f * Q + (t + 1) * 128],
                        w2s[:], start=(f == 0), stop=(f == NF - 1))
        for t in range(NQ):
            ot = streamE.tile([128, Q], F32, bufs=2, name="ot")
            if FFN2_FP8:
                nc.vector.scalar_tensor_tensor(
                    out=ot[:], in0=ops2[t][:], scalar=1.0 / SW2,
                    in1=h_sb[:, t * D + s2 * Q: t * D + (s2 + 1) * Q],
                    op0=ALU.mult, op1=ALU.add)
            else:
                nc.vector.tensor_tensor(
                    out=ot[:], in0=ops2[t][:],
                    in1=h_sb[:, t * D + s2 * Q: t * D + (s2 + 1) * Q], op=ALU.add)
            nc.sync.dma_start(out_d[t * 128:(t + 1) * 128, s2 * Q:(s2 + 1) * Q], ot[:])

    ffn2_wave(0, with_ffn1=True)
    ffn2_wave(1, with_ffn1=False)

    streamE.release()
    streamW2.release()
    poolFF.release()
    poolD.release()
    poolQKV.release()
    poolF.release()
    consts.release()
    psum.release()


_NC = None


def _get_nc():
    global _NC
    if _NC is None:
        _NC = _build()
    return _NC


def _prep_in_maps(inputs):
    x = np.asarray(inputs["x"], np.float32)          # [2, 2048, 1024]
    mask = np.asarray(inputs["mask"])                # [2, 1, 1, 2048]
    wq, bq = np.asarray(inputs["wq"], np.float32), np.asarray(inputs["bq"], np.float32)
    wk, bk = np.asarray(inputs["wk"], np.float32), np.asarray(inputs["bk"], np.float32)
    wv, bv = np.asarray(inputs["wv"], np.float32), np.asarray(inputs["bv"], np.float32)
    wo, bo = np.asarray(inputs["wo"], np.float32), np.asarray(inputs["bo"], np.float32)
    ln1_w, ln1_b = np.asarray(inputs["ln1_w"], np.float32), np.asarray(inputs["ln1_b"], np.float32)
    ln2_w, ln2_b = np.asarray(inputs["ln2_w"], np.float32), np.asarray(inputs["ln2_b"], np.float32)
    w1, b1 = np.asarray(inputs["w1"], np.float32), np.asarray(inputs["b1"], np.float32)
    w2, b2 = np.asarray(inputs["w2"], np.float32), np.asarray(inputs["b2"], np.float32)

    bf = ml_dtypes.bfloat16
    f8 = ml_dtypes.float8_e4m3
    sc = 1.0 / np.sqrt(np.float32(DK))
    wq_b = (ln1_w[:, None] * wq * sc * SQ).astype(f8)
    wk_b = (ln1_w[:, None] * wk * SK).astype(f8)
    wv_b = (ln1_w[:, None] * wv * SV).astype(f8)
    wo_b = (wo * SO).astype(f8)
    w1f = ln2_w[:, None] * w1
    if FFN1_FP8:
        w1_b = (w1f * SW1).astype(f8)
        w1r_b = (w1f * SW1 - w1_b.astype(np.float32)).astype(f8)
    else:
        w1_b = w1f.astype(bf)
        w1r_b = np.zeros_like(w1f).astype(f8)
    w2_b = (w2 * SW2).astype(f8) if FFN2_FP8 else w2.astype(bf)
    bq_v = ((ln1_b @ wq + bq) * sc).astype(np.float32)
    bk_v = (ln1_b @ wk + bk).astype(np.float32)
    bv_full = ln1_b @ wv + bv
    bo_v = (bv_full @ wo + bo).astype(np.float32)
    b1_v = (ln2_b @ w1 + b1).astype(np.float32)
    b2_v = b2.astype(np.float32)

    # selector for per-head-pair reciprocal broadcast: out rows 0:64 of pair m
    # get recip row 2m, rows 64:128 get row 2m+1
    sel = np.zeros((16, NR * 128), np.float32)
    for m in range(NR):
        sel[2 * m, m * 128: m * 128 + 64] = 1.0
        sel[2 * m + 1, m * 128 + 64: (m + 1) * 128] = 1.0
    sel_v = sel.astype(f8)

    common = dict(wq_b=wq_b, wk_b=wk_b, wv_b=wv_b, wo_b=wo_b, w1_b=w1_b,
                  w2_b=w2_b, bq_v=bq_v, bk_v=bk_v, bo_v=bo_v,
                  b1_v=b1_v, b2_v=b2_v, sel_v=sel_v)
    if FFN1_FP8:
        common["w1r_b"] = w1r_b
    in_maps = []
    for c in range(NCHIP):
        b, j = c // 4, c % 4
        q0 = j * Q
        xr = np.concatenate([x[b, q0:], x[b, :q0]], axis=0)
        mbias = np.where(mask[b, 0, 0] == 0, np.float32(-30000.0), np.float32(0.0))
        mbr = np.concatenate([mbias[q0:], mbias[:q0]]).astype(np.float32)
        in_maps.append(dict(common, x_all=np.ascontiguousarray(xr), mb_v=mbr))
    return in_maps


def kernel(**inputs):
    in_maps = _prep_in_maps(inputs)
    nc = _get_nc()
    res = run_bass_kernel_spmd(nc, in_maps, core_ids=list(range(NCHIP)))
    out = np.empty((2, S, D), np.float32)
    for c in range(NCHIP):
        b, j = c // 4, c % 4
        out[b, j * Q:(j + 1) * Q] = res.results[c]["out"]
    return out
